# revision 3
# baseline (speedup 1.0000x reference)
"""DistTokenMix kernel v2 for Trainium2 (8 NeuronCores) — fp8 DoubleRow.

Math: out[b,i,d] = sum_j h[b,j,d] * alpha[spd[i,j], d]
     (B=8, N=4096, D=64, NUM_BUCKETS=8)

As in v1: out = sum_{k<7} (alpha_k - alpha_7) * (M_k @ h) + alpha_7 * colsum(h),
with M_k[i,j] = [spd[i,j]==k]; output rows i sharded across 8 cores.

v2 speedup: fp8e4 MatmulPerfMode.DoubleRow computes A.T@mA + B.T@mB in ONE
instruction at the same per-instruction cost as bf16 (HW-verified), i.e. 2x
MACs/cycle. Two j-tiles are paired per instruction:
  - weights = h8[jt0|jt1, bd-chunk]  (e4m3 of h, host-converted)
  - moving  = mask pair [128, 2, 512] = [spd==k] in {0,1} fp8 (exact)
Pure e4m3 h gives 2.66% fro error (gate is 2e-2), so the first QJT=16 j-tiles
also stream a residual pair: weights r8 = e4m3(h - h8) stored at natural scale
(PE handles fp8 subnormals exactly, HW-verified), streaming the SAME mask as
the main pair into the same psum. End-to-end predicted error: 1.89e-2; QJT
tunable (18 -> 1.76e-2, 20 -> 1.63e-2).

Instrs/core: 7 buckets x (16 main + 8 res pairs) x 4 bd-chunks = 672 DoubleRow
matmuls (vs 896 bf16-rate in v1). Engine budget: PE ~155us busy; DVE is next
(fp8 mask gen runs 1 elem/cycle — no 2x mode for 1-byte dtypes — plus the
psum drains, which only DVE can do: GPSIMD has no PSUM access, ACT cannot
accumulate). Reliefs: ~2 masks/bucket generated on the scalar engine as
relu(1 - |v - k|), the colsum j-fold runs as chained adds on GPSIMD, csc is
applied at bucket 3 of the last pass (the GPSIMD chain is ~60us serial), two
merged psum passes (drain sets) instead of three, and the next bucket's first
mask is emitted before this bucket's drains (the in-order DVE stream would
otherwise leave the PE mask-starved at each bucket boundary).

Measured on healthy silicon: ~184us (vs 225us bf16 v1), rel err 1.889e-2.
NOTE: the device sometimes drops into a ~1.19x throttled state (all kernels
slow down equally); NEURON_RT_RESET_CORES=1 has healed it in the past.
"""
import sys
import types

import ml_dtypes
import numpy as np

import concourse.bass as bass
import concourse.mybir as mybir
import concourse.tile as tile
from concourse import bacc, bass_isa
from concourse.bass_utils import run_bass_kernel_spmd

B, N, D = 8, 4096, 64
NB = 8              # buckets
NK = NB - 1         # buckets computed via masks
NCORES = 8
IC = N // NCORES    # 512 output rows per core
BD = B * D          # 512
NJT = N // 128      # 32 j tiles
QJT = 16            # leading j-tiles that get the fp8 residual correction

f32 = mybir.dt.float32
bf16 = mybir.dt.bfloat16
f16 = mybir.dt.float16
f8 = mybir.dt.float8e4
i32 = mybir.dt.int32

# j-chunks (in j-tiles): small first chunks let the PE start early
CHUNKS = [2, 2, 4, 4, 4, 4, 4, 4, 4]
assert sum(CHUNKS) == NJT
STARTS = [sum(CHUNKS[:i]) for i in range(len(CHUNKS))]
# passes over chunk indices
PASSC = [(0, 3), (3, len(CHUNKS))]
DR = mybir.MatmulPerfMode.DoubleRow


def build_nc():
    nc = bacc.Bacc(trn_type="TRN2")
    # per-core column block of spd (transposed on host): int64 [N, IC] viewed
    # as int32 [N, 2*IC] (little-endian low word holds the value)
    spdT = nc.dram_tensor("spdT", [N, 2 * IC], i32, kind="ExternalInput")
    # h relayout [j, b*64+d] as e4m3, plus residual e4m3(h-h8) at natural
    # scale (fp8 subnormals are exact on the PE)
    h8d = nc.dram_tensor("h8d", [N, BD], f8, kind="ExternalInput")
    r8d = nc.dram_tensor("r8d", [QJT * 128, BD], f8, kind="ExternalInput")
    # alphap[p, k<7] = (alpha[k]-alpha[7])[p%64]; [p,7] = alpha[7][p%64];
    # [p,8] = 0.0625; [p, 9+k] = -k (activation-engine bias constants)
    alphap = nc.dram_tensor("alphap", [128, 16], f32, kind="ExternalInput")
    ones_d = nc.dram_tensor("ones_d", [128, 1], f16, kind="ExternalInput")
    out = nc.dram_tensor("out", [BD, IC], f32, kind="ExternalOutput")
    cs_dram = nc.dram_tensor("cs_dram", [1, BD], f32, kind="Internal")

    with tile.TileContext(nc) as tc:
        with (
            tc.tile_pool(name="persist", bufs=1) as persist,
            tc.tile_pool(name="stage", bufs=3) as stagep,
            tc.tile_pool(name="maskp", bufs=8) as maskp,
            tc.tile_pool(name="psum", bufs=2, space="PSUM") as psump,
        ):
            # ---- persistent tiles (chunked for fine-grained deps) ----
            h_sb = [persist.tile([128, nt, BD], f8, name=f"h{c}", tag=f"h{c}")
                    for c, nt in enumerate(CHUNKS)]
            # residual tiles for corrected chunks (leading QJT j-tiles)
            r_sb = {}
            for c, nt in enumerate(CHUNKS):
                if STARTS[c] < QJT:
                    assert STARTS[c] + nt <= QJT
                    r_sb[c] = persist.tile([128, nt, BD], f8,
                                           name=f"r{c}", tag=f"r{c}")
            vals = [persist.tile([128, nt, IC], bf16, name=f"v{c}", tag=f"v{c}")
                    for c, nt in enumerate(CHUNKS)]
            beta = persist.tile([128, 16], f32)
            ones = persist.tile([128, 1], f16)
            accs = [persist.tile([128, IC], f32, name=f"acc{c}", tag=f"acc{c}")
                    for c in range(4)]

            spd_t = spdT.rearrange("(t p) w -> t p w", p=128)
            h_t = h8d.rearrange("(t p) w -> t p w", p=128)
            r_t = r8d.rearrange("(t p) w -> t p w", p=128)

            # ---- interleaved spd/h chunk loads; compact spd to bf16 ----
            nc.sync.dma_start(ones[:], ones_d[:])
            nc.sync.dma_start(beta[:], alphap[:])
            for c, nt in enumerate(CHUNKS):
                t0 = STARTS[c]
                stage = stagep.tile([128, 4, 2 * IC], i32, name="stage",
                                    tag="stage")
                nc.sync.dma_start(stage[:, 0:nt, :],
                                  spd_t[t0:t0 + nt].rearrange("t p w -> p t w"))
                nc.sync.dma_start(h_sb[c][:],
                                  h_t[t0:t0 + nt].rearrange("t p w -> p t w"))
                if c in r_sb:
                    nc.sync.dma_start(r_sb[c][:],
                                      r_t[t0:t0 + nt].rearrange("t p w -> p t w"))
                # low int32 of each int64 -> bf16 (values 0..7, exact)
                nc.scalar.copy(vals[c][:], stage[:, 0:nt, ::2])

            # (b) HAM warmup: junk matmuls on the ones column while DMA runs
            wps = psump.tile([1, 512], f32, name="wps", tag="ps3")
            for _ in range(12):
                nc.tensor.matmul(wps[:, 0:8], ones[:],
                                 ones[:, 0:1].broadcast_to((128, 8)),
                                 start=True, stop=True)
            for _ in range(10):
                nc.tensor.matmul(wps[:], ones[:],
                                 ones[:, 0:1].broadcast_to((128, 512)),
                                 start=True, stop=True)

            # colsum off the tensor engine: chained adds on GPSIMD fold every
            # h8 j-tile (+ r8 j-tiles, stored at natural scale), then one
            # GPSIMD partition all-reduce folds the 128 j-residues.
            acc_cs = persist.tile([128, BD], f32)
            tiles = [h_sb[c][:, t] for c, nt in enumerate(CHUNKS)
                     for t in range(nt)]
            tiles += [r_sb[c][:, t] for c in sorted(r_sb)
                      for t in range(CHUNKS[c])]
            nc.gpsimd.tensor_tensor(acc_cs[:], tiles[0], tiles[1],
                                    mybir.AluOpType.add)
            for tl in tiles[2:]:
                nc.gpsimd.tensor_tensor(acc_cs[:], acc_cs[:], tl,
                                        mybir.AluOpType.add)
            cs_all = persist.tile([128, BD], f32)
            nc.gpsimd.partition_all_reduce(
                cs_all[:], acc_cs[:], 128, bass_isa.ReduceOp.add,
            )

            # ---- bucket x pass DoubleRow matmul loop ----
            csc = persist.tile([128, 4], f32)
            csc_added = False
            for pi, (ca, cb) in enumerate(PASSC):
                if pi == len(PASSC) - 1:
                    # colsum(h) before the last pass: reshape [1,BD] to the
                    # per-partition layout via a DRAM round-trip that hides
                    # under the matmuls.
                    cs_sb = cs_all[0:1, :]
                    nc.sync.dma_start(cs_dram[:], cs_sb[:])
                    cs_part = persist.tile([128, 4], f32)
                    nc.sync.dma_start(
                        cs_part[:],
                        cs_dram.rearrange("x (c p) -> (x p) c", p=128),
                    )
                    nc.vector.tensor_scalar(
                        csc[:], cs_part[:], beta[:, NB - 1:NB], None,
                        mybir.AluOpType.mult,
                    )
                # pairs (chunk, local pair idx) in this pass
                pairs = [(c, u) for c in range(ca, cb)
                         for u in range(CHUNKS[c] // 2)]
                # matmuls per (k, ch) group in this pass: mains + residuals
                ngrp = len(pairs) + sum(1 for (c, u) in pairs if c in r_sb)
                prefetched = {}
                for k in range(NK):
                    last = (pi == len(PASSC) - 1 and k == NK - 1)
                    pss = [psump.tile([128, IC], f32, name=f"ps{ch}",
                                      tag=f"ps{ch}") for ch in range(4)]
                    seen = {ch: 0 for ch in range(4)}

                    def mm(wt_tile, lt, mask3, ch):
                        seen[ch] += 1
                        nc.tensor.matmul(
                            pss[ch][:],
                            wt_tile[:, lt:lt + 2, ch * 128:(ch + 1) * 128],
                            mask3,
                            start=(seen[ch] == 1),
                            stop=(seen[ch] == ngrp),
                            perf_mode=DR,
                        )

                    def gen_mask(kk, pj, c, u, force_dve=False):
                        lt = 2 * u
                        mask = maskp.tile([128, 2, IC], f8, name="mask",
                                          tag="mask")
                        if (pi == len(PASSC) - 1 and pj % 5 == 4
                                and not force_dve):
                            # offload some mask gen to the idle scalar engine:
                            # t = |v - k|; mask = relu(1 - t)  ->  {0, 1}
                            tq = maskp.tile([128, 2, IC], f16, name="tq",
                                            tag="tq")
                            nc.scalar.activation(
                                tq[:], vals[c][:, lt:lt + 2, :],
                                mybir.ActivationFunctionType.Abs,
                                bias=beta[:, 9 + kk:10 + kk], scale=1.0,
                            )
                            nc.scalar.activation(
                                mask[:], tq[:],
                                mybir.ActivationFunctionType.Relu,
                                bias=1.0, scale=-1.0,
                            )
                        else:
                            nc.vector.tensor_scalar(
                                mask[:], vals[c][:, lt:lt + 2, :], float(kk),
                                None, mybir.AluOpType.is_equal,
                            )
                        return mask

                    def drain(ch):
                        deng = nc.vector
                        if pi == 0 and k == 0:
                            deng.tensor_scalar(
                                accs[ch][:], pss[ch][:], beta[:, k:k + 1],
                                None, mybir.AluOpType.mult,
                            )
                        else:
                            deng.scalar_tensor_tensor(
                                accs[ch][:], pss[ch][:], beta[:, k:k + 1],
                                accs[ch][:],
                                op0=mybir.AluOpType.mult,
                                op1=mybir.AluOpType.add,
                            )
                        if pi == len(PASSC) - 1 and k == 3 and not csc_added:
                            deng.tensor_scalar(
                                accs[ch][:], accs[ch][:], csc[:, ch:ch + 1],
                                None, mybir.AluOpType.add,
                            )
                        if last:
                            eng = nc.sync if ch % 2 == 0 else nc.scalar
                            eng.dma_start(out[ch * 128:(ch + 1) * 128, :],
                                          accs[ch][:])

                    cur = prefetched.pop(k, {})
                    for pj, (c, u) in enumerate(pairs):
                        mask = cur.get(pj)
                        if mask is None:
                            mask = gen_mask(k, pj, c, u)
                        lt = 2 * u
                        for ch in range(4):
                            mm(h_sb[c], lt, mask[:], ch)
                        if c in r_sb:
                            # residual pair reuses the same mask (r8 is
                            # stored at natural scale)
                            for ch in range(4):
                                mm(r_sb[c], lt, mask[:], ch)
                    # pre-generate the next bucket's first masks BEFORE the
                    # drains: the DVE stream is in-order, and the drains block
                    # on this bucket's psum stop — without the hoist the PE
                    # enters bucket k+1 with no masks ready (~1.4us stall per
                    # bucket boundary)
                    if k + 1 < NK:
                        nxt = {}
                        for pj in range(min(1, len(pairs))):
                            c, u = pairs[pj]
                            nxt[pj] = gen_mask(k + 1, pj, c, u,
                                               force_dve=True)
                        prefetched[k + 1] = nxt
                    for ch in range(4):
                        drain(ch)
                    if pi == len(PASSC) - 1 and k == 3:
                        csc_added = True

    nc.compile()
    return nc


def _ensure_ntff_hook_module():
    """bass_utils imports antenv.axon_hooks when tracing is requested; the
    image may lack it. Provide a no-op registry so trace requests degrade
    to plain execution instead of crashing."""
    try:
        import antenv.axon_hooks  # noqa: F401
        return
    except ImportError:
        pass
    try:
        import antenv
    except ImportError:
        return
    mod = types.ModuleType("antenv.axon_hooks")
    mod._hook = None
    mod.set_axon_ntff_profile_hook = lambda h: setattr(mod, "_hook", h)
    mod.get_axon_ntff_profile_hook = lambda: mod._hook
    sys.modules["antenv.axon_hooks"] = mod
    antenv.axon_hooks = mod
    try:
        from trn_agent_boot.trn_boot import _ntff_profile_via_ctypes
        mod._hook = _ntff_profile_via_ctypes("/opt/axon/libaxon_pjrt.so")
    except Exception:
        pass


_ensure_ntff_hook_module()

_NC_CACHE = None


def _get_nc():
    global _NC_CACHE
    if _NC_CACHE is None:
        _NC_CACHE = build_nc()
    return _NC_CACHE


def _prep_inputs(h, spd, alpha):
    h = np.asarray(h, dtype=np.float32)
    alpha = np.asarray(alpha, dtype=np.float32)
    spd = np.asarray(spd)
    if spd.dtype != np.int64:
        spd = spd.astype(np.int64)
    # [j, b*64+d]
    h_jbd = np.ascontiguousarray(h.transpose(1, 0, 2).reshape(N, BD))
    h8 = h_jbd.astype(ml_dtypes.float8_e4m3)
    r = h_jbd - h8.astype(np.float32)
    r8 = np.ascontiguousarray(r[:QJT * 128].astype(ml_dtypes.float8_e4m3))
    pidx = np.arange(128) % 64
    alphap = np.zeros((128, 16), dtype=np.float32)
    for k in range(NK):
        alphap[:, k] = (alpha[k] - alpha[NB - 1])[pidx]
        alphap[:, 9 + k] = -float(k)
    alphap[:, NB - 1] = alpha[NB - 1][pidx]
    alphap[:, 8] = 0.0625
    alphap = np.ascontiguousarray(alphap)
    in_maps = []
    for c in range(NCORES):
        blk = np.ascontiguousarray(spd[c * IC:(c + 1) * IC, :].T)  # [N, IC] int64
        spdT_i32 = blk.view(np.int32).reshape(N, 2 * IC)
        in_maps.append({"spdT": spdT_i32, "h8d": h8, "r8d": r8,
                        "alphap": alphap,
                        "ones_d": np.ones((128, 1), dtype=np.float16)})
    return in_maps


def _assemble(results):
    outs = []
    for c in range(NCORES):
        o = results[c]["out"]                       # [BD, IC] = [(b,d), i]
        outs.append(o.reshape(B, D, IC).transpose(0, 2, 1))  # [b, i_local, d]
    return np.ascontiguousarray(np.concatenate(outs, axis=1))  # [B, N, D]


def kernel(h, spd, alpha, _trace=False):
    nc = _get_nc()
    in_maps = _prep_inputs(h, spd, alpha)
    res = run_bass_kernel_spmd(nc, in_maps, list(range(NCORES)), trace=_trace)
    out = _assemble(res.results)
    if _trace:
        kernel.last_result = res
    return out
